# revision 22
# baseline (speedup 1.0000x reference)
"""Multi-head attention, tensor-parallel across 8 Trainium2 NeuronCores.

Sharding: core = (batch b, head-group g) with b in {0,1}, g in {0..3}.
Each core computes 4 heads (a 256-wide slice of the head dimension) for one
batch element:
  Q^T/K^T = Wq/Wk slice^T-projections of query/key (kept transposed: [dh, s])
  V       = value @ Wv slice (natural [s, dh]), with an appended ones column
  S^T     = K^T-chunk.T @ Q^T-chunk per head  -> scores transposed [j, i]
  E       = exp(S^T * scale)                  (no max subtraction; scores ~N(0,1))
  [O^T;Z] = V'.T @ E  accumulated over j      (ones column yields Z = sum_j E)
  Onorm^T = O^T * (1/Z) broadcast
  outT    = Wo-slice.T @ Onorm^T (+ bo on group-0 cores only)
Host: transposes activations into [D, S] per core, and sums the 4 group
partials per batch (the "all-reduce" of the output projection), then
transposes back.

Inputs arrive full-size; all sharding is internal.
"""

import numpy as np

# Problem shape (hardcoded per the harness contract).
B, S, D, H = 2, 2048, 1024, 16
DK = D // H              # 64 head dim
N_CORES = 8
GROUPS = N_CORES // B    # 4 head-groups
DH = D // GROUPS         # 256 head-dims per core (4 heads)
H_CORE = DH // DK        # 4 heads per core
SCALE = 1.0 / float(np.sqrt(DK))

P = 128                  # SBUF/PSUM partitions
SC = 512                 # matmul moving-dim chunk (one PSUM bank of fp32)
IB = 1024                # flash i-block (exp granule)


def build_nc(S=S, D=D, DH=DH, DK=DK, scale=SCALE, ib=IB, dtype="f32r"):
    """Build the per-core Bass module (same NEFF for all 8 cores)."""
    import concourse.bacc as bacc
    import concourse.mybir as mybir
    import concourse.tile as tile

    f32 = mybir.dt.float32
    f32r = mybir.dt.float32r
    bf16 = mybir.dt.bfloat16
    Exp = mybir.ActivationFunctionType.Exp

    KT = D // P                    # contraction tiles for projections
    NSC = S // SC                  # s chunks
    HC = DH // P                   # head-dim chunks (2)
    HPC = P // DK                  # heads per chunk (2)
    H_CORE = DH // DK
    JT = S // P                    # j tiles
    NIB = S // ib                  # i blocks
    ICB = ib // SC                 # i chunks per block
    NOUT = D // P                  # output row chunks

    cdt = {"f32r": f32r, "bf16": bf16, "f32": f32}[dtype]

    def mm(ap):
        return ap

    nc = bacc.Bacc("TRN2", target_bir_lowering=False, debug=False)

    qT = nc.dram_tensor("qT", [D, S], cdt, kind="ExternalInput")
    kTd = nc.dram_tensor("kTd", [D, S], cdt, kind="ExternalInput")
    vT = nc.dram_tensor("vT", [D, S], cdt, kind="ExternalInput")
    wq = nc.dram_tensor("wq", [D, DH], cdt, kind="ExternalInput")
    wk = nc.dram_tensor("wk", [D, DH], cdt, kind="ExternalInput")
    wv = nc.dram_tensor("wv", [D, DH], cdt, kind="ExternalInput")
    wo = nc.dram_tensor("wo", [DH, D], cdt, kind="ExternalInput")
    bq = nc.dram_tensor("bq", [P, HC], f32, kind="ExternalInput")
    bk = nc.dram_tensor("bk", [P, HC], f32, kind="ExternalInput")
    bvb = nc.dram_tensor("bvb", [P, H_CORE, DK], f32, kind="ExternalInput")
    bo = nc.dram_tensor("bo", [P, NOUT], f32, kind="ExternalInput")
    outT = nc.dram_tensor("outT", [D, S], f32, kind="ExternalOutput")

    with tile.TileContext(nc) as tc:
        with (
            tc.tile_pool(name="const", bufs=1) as cpool,
            tc.tile_pool(name="pers", bufs=1) as pers,
            tc.tile_pool(name="stream", bufs=1) as stream,
            tc.tile_pool(name="psum", bufs=1, space="PSUM") as psum,
            tc.tile_pool(name="dscratch", bufs=1, space="DRAM") as dscratch,
        ):
            # ---- constants ----
            wq_sb = cpool.tile([P, KT, DH], cdt, name="wq_sb")
            wk_sb = cpool.tile([P, KT, DH], cdt, name="wk_sb")
            wv_sb = cpool.tile([P, KT, DH], cdt, name="wv_sb")
            wo_sb = cpool.tile([P, HC, D], cdt, name="wo_sb")
            bq_sb = cpool.tile([P, HC], f32, name="bq_sb")
            bk_sb = cpool.tile([P, HC], f32, name="bk_sb")
            bvb_sb = cpool.tile([P, H_CORE, DK], f32, name="bvb_sb")
            bo_sb = cpool.tile([P, NOUT], f32, name="bo_sb")
            nc.sync.dma_start(wq_sb[:], qT_ap_rearr(wq, P))
            nc.sync.dma_start(wk_sb[:], qT_ap_rearr(wk, P))
            nc.sync.dma_start(wv_sb[:], qT_ap_rearr(wv, P))
            nc.sync.dma_start(wo_sb[:], wo[:, :].rearrange("(c p) n -> p c n", p=P))
            nc.sync.dma_start(bq_sb[:], bq[:, :])
            nc.sync.dma_start(bk_sb[:], bk[:, :])
            nc.sync.dma_start(bvb_sb[:], bvb[:, :, :])
            nc.sync.dma_start(bo_sb[:], bo[:, :])

            # ---- persistent activations ----
            # Q^T/K^T live per head on partitions 64-127 (base-64 K=64
            # matmuls sustain full rate; base-0 ones run at half rate).
            qt_h = [pers.tile([P, S], cdt, name=f"qth{h}")
                    for h in range(H_CORE)]
            kt_h = [pers.tile([P, S], cdt, name=f"kth{h}")
                    for h in range(H_CORE)]
            v_c = [pers.tile([P, JT, HPC, DK + 1], cdt, name=f"v{c}") for c in range(HC)]
            on_c = [pers.tile([P, S], cdt, name=f"on{c}") for c in range(HC)]

            for c in range(HC):
                ones_ap = v_c[c][:, :, :, DK:DK + 1]
                if dtype == "f32r":
                    ones_ap = ones_ap.bitcast(f32)
                nc.vector.memset(ones_ap, 1.0)

            # ---- projections ----
            def qk_proj(src, w_sb, b_sb, dst, chunks):
                for si in range(NSC):
                    ins = []
                    for kt in range(KT):
                        t = stream.tile([P, SC], cdt, tag="instream", bufs=12,
                                        name=f"in_{src.name}_{si}_{kt}_{chunks[0]}")
                        nc.sync.dma_start(
                            t[:], src[kt * P:(kt + 1) * P,
                                      si * SC:(si + 1) * SC])
                        ins.append(t)
                        yield
                    for c in chunks:
                        ps = psum.tile([P, SC], f32, tag="mm", bufs=4,
                                       name=f"ps_{src.name}_{si}_{c}")
                        for kt in range(KT):
                            nc.tensor.matmul(
                                ps[:],
                                lhsT=mm(w_sb[:, kt, c * P:(c + 1) * P]),
                                rhs=mm(ins[kt][:]),
                                start=(kt == 0), stop=(kt == KT - 1))
                            yield
                        ssl = slice(si * SC, (si + 1) * SC)
                        # odd head: psum rows 64-127 are already partition-
                        # aligned with its home; write directly.
                        nc.vector.tensor_add(
                            dst[c * HPC + 1][DK:P, ssl], ps[DK:P, :],
                            b_sb[DK:P, c:c + 1].to_broadcast((DK, SC)))
                        stg = stream.tile([P, SC], cdt, tag="pstage", bufs=3,
                                          name=f"stg_{src.name}_{si}_{c}")
                        nc.vector.tensor_add(
                            stg[0:DK, :], ps[0:DK, :],
                            b_sb[0:DK, c:c + 1].to_broadcast((DK, SC)))
                        nc.sync.dma_start(dst[c * HPC][DK:P, ssl],
                                          stg[0:DK, :])
                        yield

            for g in (qk_proj(qT, wq_sb, bq_sb, qt_h, (0,)),
                      qk_proj(kTd, wk_sb, bk_sb, kt_h, (0,))):
                for _ in g:
                    pass
            pending = ([qk_proj(qT, wq_sb, bq_sb, qt_h, (1,)),
                        qk_proj(kTd, wk_sb, bk_sb, kt_h, (1,))]
                       if HC > 1 else [])

            # V natural: psum[s, dh] = sum_k vT[k, s] * Wv[k, dh]
            for si in range(NSC):
                ins = []
                for kt in range(KT):
                    t = stream.tile([P, SC], cdt, tag="instream", bufs=12,
                                    name=f"in_v_{si}_{kt}")
                    nc.sync.dma_start(
                        t[:], vT[kt * P:(kt + 1) * P, si * SC:(si + 1) * SC])
                    ins.append(t)
                for sub in range(SC // P):
                    jt_idx = si * (SC // P) + sub
                    ps = psum.tile([P, DH], f32, tag="mm", bufs=4,
                                   name=f"ps_v_{jt_idx}")
                    for kt in range(KT):
                        nc.tensor.matmul(
                            ps[:],
                            lhsT=mm(ins[kt][:, sub * P:(sub + 1) * P]),
                            rhs=mm(wv_sb[:, kt, :]),
                            start=(kt == 0), stop=(kt == KT - 1))
                    for c in range(HC):
                        nc.vector.tensor_add(
                            v_c[c][:, jt_idx, :, 0:DK],
                            ps[:, c * P:(c + 1) * P].rearrange(
                                "p (h d) -> p h d", d=DK),
                            bvb_sb[:, c * HPC:(c + 1) * HPC, :])

            # ---- attention (flash over j, scores transposed) ----
            # Per-head blocks; sc has two buffers so scores(jt+1) overlap
            # exp(jt). AV matmuls trail one j-step so the PE program never
            # blocks the ACT engine behind unready work.
            blk = 0
            for h in range(H_CORE):
                hc = h // HPC
                hh = h % HPC
                p0 = hh * DK
                for ibx in range(NIB):
                    # between early flash blocks, emit one chunk-1 projection
                    # sweep so it executes under the previous block's exps
                    if blk in (1, 2) and pending:
                        for _ in pending.pop(0):
                            pass
                    blk += 1
                    i0 = ibx * ib
                    avs = [
                        psum.tile([P, SC], f32, tag="mm", bufs=4,
                                  name=f"av_{h}_{ibx}_{ic}")
                        for ic in range(ICB)
                    ]
                    e_ts = {}
                    for jt in range(JT + 1):
                        if jt < JT:
                            sc_t = psum.tile([P, ib], f32, tag="sc",
                                             bufs=2,
                                             name=f"sc_{h}_{ibx}_{jt}")
                            for ic in range(ICB):
                                nc.tensor.matmul(
                                    sc_t[:, ic * SC:(ic + 1) * SC],
                                    lhsT=mm(kt_h[h][DK:P,
                                                    jt * P:(jt + 1) * P]),
                                    rhs=mm(qt_h[h][DK:P,
                                                   i0 + ic * SC:i0 + (ic + 1) * SC]),
                                    start=True, stop=True)
                            e_t = stream.tile([P, ib], cdt, tag="e", bufs=3,
                                              name=f"e_{h}_{ibx}_{jt}")
                            nc.scalar.activation(e_t[:], sc_t[:], Exp,
                                                 bias=0.0, scale=scale)
                            e_ts[jt] = e_t
                        if jt >= 1:
                            pj = jt - 1
                            e_t = e_ts.pop(pj)
                            for ic in range(ICB):
                                nc.tensor.matmul(
                                    avs[ic][0:DK + 1, :],
                                    lhsT=mm(v_c[hc][:, pj, hh, :]),
                                    rhs=mm(e_t[:, ic * SC:(ic + 1) * SC]),
                                    start=(pj == 0), stop=(pj == JT - 1))
                    # drain AV psums to SBUF, normalize in the background
                    for ic in range(ICB):
                        av = avs[ic]
                        av_sb = stream.tile([P, SC], f32, tag="avsb", bufs=4,
                                            name=f"avsb_{h}_{ibx}_{ic}")
                        nc.vector.tensor_copy(av_sb[0:DK + 1, :],
                                              av[0:DK + 1, :])
                        rz = stream.tile([P, SC], f32, tag="rz", bufs=2,
                                         name=f"rz_{h}_{ibx}_{ic}")
                        nc.vector.reciprocal(rz[DK:DK + 1, :],
                                             av_sb[DK:DK + 1, :])
                        rz_d = dscratch.tile([1, SC], f32, tag="rzd", bufs=2,
                                             name=f"rzd_{h}_{ibx}_{ic}")
                        nc.sync.dma_start(rz_d[:], rz[DK:DK + 1, :])
                        rzb = stream.tile([P, SC], f32, tag="rzb", bufs=2,
                                          name=f"rzb_{h}_{ibx}_{ic}")
                        nc.sync.dma_start(
                            rzb[0:DK, :],
                            rz_d[:, :].to_broadcast((DK, SC)))
                        ot = stream.tile([P, SC], cdt, tag="ot", bufs=2,
                                         name=f"ot_{h}_{ibx}_{ic}")
                        nc.vector.tensor_mul(ot[0:DK, :], av_sb[0:DK, :],
                                             rzb[0:DK, :])
                        nc.sync.dma_start(
                            on_c[hc][p0:p0 + DK,
                                     i0 + ic * SC:i0 + (ic + 1) * SC],
                            ot[0:DK, :])

            # ---- output projection ----
            Ident = mybir.ActivationFunctionType.Identity
            for n in range(NOUT):
                for i in range(NSC):
                    idx = n * NSC + i
                    ps = psum.tile([P, SC], f32, tag=("sc", "mm")[idx % 2],
                                   bufs=(2, 4)[idx % 2],
                                   name=f"ps_o_{n}_{i}")
                    for c in range(HC):
                        nc.tensor.matmul(
                            ps[:],
                            lhsT=mm(wo_sb[:, c, n * P:(n + 1) * P]),
                            rhs=mm(on_c[c][:, i * SC:(i + 1) * SC]),
                            start=(c == 0), stop=(c == HC - 1))
                    o_sb = stream.tile([P, SC], f32, tag="osb", bufs=4,
                                       name=f"o_sb_{n}_{i}")
                    if idx % 2 == 0:
                        nc.scalar.activation(o_sb[:], ps[:], Ident,
                                             bias=bo_sb[:, n:n + 1],
                                             scale=1.0)
                    else:
                        nc.vector.tensor_add(
                            o_sb[:], ps[:],
                            bo_sb[:, n:n + 1].to_broadcast((P, SC)))
                    nc.sync.dma_start(
                        outT[n * P:(n + 1) * P, i * SC:(i + 1) * SC], o_sb[:])

    nc.finalize()
    return nc


def qT_ap_rearr(w_dram, p):
    """[D, N] dram weight -> [P, D//P, N] AP for SBUF load."""
    return w_dram[:, :].rearrange("(ko p) n -> p ko n", p=p)


def make_in_maps(query, key, value, Wq, bq, Wk, bk, Wv, bv, Wo, bo,
                 dtype="f32r"):
    """Shard full inputs into the 8 per-core input dicts."""
    f = lambda a: np.ascontiguousarray(np.asarray(a, dtype=np.float32))
    HC = DH // P
    NOUT = D // P
    query, key, value = f(query), f(key), f(value)
    Wq, Wk, Wv, Wo = f(Wq), f(Wk), f(Wv), f(Wo)
    bq, bk, bv, bo = f(bq), f(bk), f(bv), f(bo)
    if dtype == "bf16":
        import ml_dtypes
        cvt = lambda a: np.ascontiguousarray(a.astype(ml_dtypes.bfloat16))
    else:
        cvt = np.ascontiguousarray
    in_maps = []
    for core in range(N_CORES):
        b, g = core // GROUPS, core % GROUPS
        sl = slice(g * DH, (g + 1) * DH)
        in_maps.append({
            "qT": cvt(query[b].T),
            "kTd": cvt(key[b].T),
            "vT": cvt(value[b].T),
            "wq": cvt(Wq[:, sl]),
            "wk": cvt(Wk[:, sl]),
            "wv": cvt(Wv[:, sl]),
            "wo": cvt(Wo[sl, :]),
            "bq": np.ascontiguousarray(bq[sl].reshape(HC, P).T),
            "bk": np.ascontiguousarray(bk[sl].reshape(HC, P).T),
            "bvb": np.ascontiguousarray(
                np.broadcast_to(bv[sl].reshape(H_CORE, DK)[None], (P, H_CORE, DK))),
            "bo": (np.ascontiguousarray(bo.reshape(NOUT, P).T)
                   if g == 0 else np.zeros((P, NOUT), np.float32)),
        })
    return in_maps


# test hooks (ignored by the harness)
TRACE = False
LAST_RESULT = None
DTYPE = "bf16"
_NC_CACHE = {}


def kernel(query, key, value, Wq, bq, Wk, bk, Wv, bv, Wo, bo):
    global LAST_RESULT
    from concourse.bass_utils import run_bass_kernel_spmd

    if DTYPE not in _NC_CACHE:
        _NC_CACHE[DTYPE] = build_nc(dtype=DTYPE)
    nc = _NC_CACHE[DTYPE]

    in_maps = make_in_maps(query, key, value, Wq, bq, Wk, bk, Wv, bv, Wo, bo,
                           dtype=DTYPE)
    kwargs = {}
    if TRACE:
        kwargs = dict(trace=True, trace_cores=[0])
    res = run_bass_kernel_spmd(nc, in_maps, core_ids=list(range(N_CORES)), **kwargs)
    LAST_RESULT = res

    out = np.zeros((B, S, D), np.float32)
    for core in range(N_CORES):
        b = core // GROUPS
        out[b] += res.results[core]["outT"].T
    return out


# revision 23
# speedup vs baseline: 1.1891x; 1.1891x over previous
"""Multi-head attention, tensor-parallel across 8 Trainium2 NeuronCores.

Sharding: core = (batch b, head-group g) with b in {0,1}, g in {0..3}.
Each core computes 4 heads (a 256-wide slice of the head dimension) for one
batch element:
  Q^T/K^T = Wq/Wk slice^T-projections of query/key (kept transposed: [dh, s])
  V       = value @ Wv slice (natural [s, dh]), with an appended ones column
  S^T     = K^T-chunk.T @ Q^T-chunk per head  -> scores transposed [j, i]
  E       = exp(S^T * scale)                  (no max subtraction; scores ~N(0,1))
  [O^T;Z] = V'.T @ E  accumulated over j      (ones column yields Z = sum_j E)
  Onorm^T = O^T * (1/Z) broadcast
  outT    = Wo-slice.T @ Onorm^T (+ bo on group-0 cores only)
Host: transposes activations into [D, S] per core, and sums the 4 group
partials per batch (the "all-reduce" of the output projection), then
transposes back.

Inputs arrive full-size; all sharding is internal.
"""

import numpy as np

# Problem shape (hardcoded per the harness contract).
B, S, D, H = 2, 2048, 1024, 16
DK = D // H              # 64 head dim
N_CORES = 8
GROUPS = N_CORES // B    # 4 head-groups
DH = D // GROUPS         # 256 head-dims per core (4 heads)
H_CORE = DH // DK        # 4 heads per core
SCALE = 1.0 / float(np.sqrt(DK))

P = 128                  # SBUF/PSUM partitions
SC = 512                 # matmul moving-dim chunk (one PSUM bank of fp32)
IB = 1024                # flash i-block (exp granule)


def build_nc(S=S, D=D, DH=DH, DK=DK, scale=SCALE, ib=IB, dtype="f32r"):
    """Build the per-core Bass module (same NEFF for all 8 cores)."""
    import concourse.bacc as bacc
    import concourse.mybir as mybir
    import concourse.tile as tile

    f32 = mybir.dt.float32
    f32r = mybir.dt.float32r
    bf16 = mybir.dt.bfloat16
    Exp = mybir.ActivationFunctionType.Exp

    KT = D // P                    # contraction tiles for projections
    NSC = S // SC                  # s chunks
    HC = DH // P                   # head-dim chunks (2)
    HPC = P // DK                  # heads per chunk (2)
    H_CORE = DH // DK
    JT = S // P                    # j tiles
    NIB = S // ib                  # i blocks
    ICB = ib // SC                 # i chunks per block
    NOUT = D // P                  # output row chunks

    cdt = {"f32r": f32r, "bf16": bf16, "f32": f32}[dtype]

    def mm(ap):
        return ap

    nc = bacc.Bacc("TRN2", target_bir_lowering=False, debug=False)

    qT = nc.dram_tensor("qT", [D, S], cdt, kind="ExternalInput")
    kTd = nc.dram_tensor("kTd", [D, S], cdt, kind="ExternalInput")
    vT = nc.dram_tensor("vT", [D, S], cdt, kind="ExternalInput")
    wq = nc.dram_tensor("wq", [D, DH], cdt, kind="ExternalInput")
    wk = nc.dram_tensor("wk", [D, DH], cdt, kind="ExternalInput")
    wv = nc.dram_tensor("wv", [D, DH], cdt, kind="ExternalInput")
    wo = nc.dram_tensor("wo", [DH, D], cdt, kind="ExternalInput")
    bq = nc.dram_tensor("bq", [P, HC], f32, kind="ExternalInput")
    bk = nc.dram_tensor("bk", [P, HC], f32, kind="ExternalInput")
    bvb = nc.dram_tensor("bvb", [P, H_CORE, DK], f32, kind="ExternalInput")
    bo = nc.dram_tensor("bo", [P, NOUT], f32, kind="ExternalInput")
    outT = nc.dram_tensor("outT", [D, S], f32, kind="ExternalOutput")

    with tile.TileContext(nc) as tc:
        with (
            tc.tile_pool(name="const", bufs=1) as cpool,
            tc.tile_pool(name="pers", bufs=1) as pers,
            tc.tile_pool(name="stream", bufs=1) as stream,
            tc.tile_pool(name="psum", bufs=1, space="PSUM") as psum,
            tc.tile_pool(name="dscratch", bufs=1, space="DRAM") as dscratch,
        ):
            # ---- constants ----
            wq_sb = cpool.tile([P, KT, DH], cdt, name="wq_sb")
            wk_sb = cpool.tile([P, KT, DH], cdt, name="wk_sb")
            wv_sb = cpool.tile([P, KT, DH], cdt, name="wv_sb")
            wo_sb = cpool.tile([P, HC, D], cdt, name="wo_sb")
            bq_sb = cpool.tile([P, HC], f32, name="bq_sb")
            bk_sb = cpool.tile([P, HC], f32, name="bk_sb")
            bvb_sb = cpool.tile([P, H_CORE, DK], f32, name="bvb_sb")
            bo_sb = cpool.tile([P, NOUT], f32, name="bo_sb")
            nc.sync.dma_start(wq_sb[:], qT_ap_rearr(wq, P))
            nc.sync.dma_start(wk_sb[:], qT_ap_rearr(wk, P))
            nc.sync.dma_start(wv_sb[:], qT_ap_rearr(wv, P))
            nc.sync.dma_start(wo_sb[:], wo[:, :].rearrange("(c p) n -> p c n", p=P))
            nc.sync.dma_start(bq_sb[:], bq[:, :])
            nc.sync.dma_start(bk_sb[:], bk[:, :])
            nc.sync.dma_start(bvb_sb[:], bvb[:, :, :])
            nc.sync.dma_start(bo_sb[:], bo[:, :])

            # ---- persistent activations ----
            # Q^T/K^T live per head on partitions 64-127 (base-64 K=64
            # matmuls sustain full rate; base-0 ones run at half rate).
            qt_h = [pers.tile([P, S], cdt, name=f"qth{h}")
                    for h in range(H_CORE)]
            kt_h = [pers.tile([P, S], cdt, name=f"kth{h}")
                    for h in range(H_CORE)]
            v_c = [pers.tile([P, JT, HPC, DK + 1], cdt, name=f"v{c}") for c in range(HC)]
            on_c = [pers.tile([P, S], cdt, name=f"on{c}") for c in range(HC)]

            for c in range(HC):
                ones_ap = v_c[c][:, :, :, DK:DK + 1]
                if dtype == "f32r":
                    ones_ap = ones_ap.bitcast(f32)
                nc.vector.memset(ones_ap, 1.0)

            # ---- projections ----
            def qk_proj(src, w_sb, b_sb, dst, chunks):
                for si in range(NSC):
                    ins = []
                    for kt in range(KT):
                        t = stream.tile([P, SC], cdt, tag="instream", bufs=12,
                                        name=f"in_{src.name}_{si}_{kt}_{chunks[0]}")
                        nc.sync.dma_start(
                            t[:], src[kt * P:(kt + 1) * P,
                                      si * SC:(si + 1) * SC])
                        ins.append(t)
                        yield
                    for c in chunks:
                        ps = psum.tile([P, SC], f32, tag="mm", bufs=4,
                                       name=f"ps_{src.name}_{si}_{c}")
                        for kt in range(KT):
                            nc.tensor.matmul(
                                ps[:],
                                lhsT=mm(w_sb[:, kt, c * P:(c + 1) * P]),
                                rhs=mm(ins[kt][:]),
                                start=(kt == 0), stop=(kt == KT - 1))
                            yield
                        stg = stream.tile([P, SC], cdt, tag="pstage", bufs=3,
                                          name=f"stg_{src.name}_{si}_{c}")
                        nc.vector.tensor_add(
                            stg[:], ps[:],
                            b_sb[:, c:c + 1].to_broadcast((P, SC)))
                        ssl = slice(si * SC, (si + 1) * SC)
                        nc.sync.dma_start(dst[c * HPC][DK:P, ssl],
                                          stg[0:DK, :])
                        nc.sync.dma_start(dst[c * HPC + 1][DK:P, ssl],
                                          stg[DK:P, :])
                        yield

            for g in (qk_proj(qT, wq_sb, bq_sb, qt_h, tuple(range(HC))),
                      qk_proj(kTd, wk_sb, bk_sb, kt_h, tuple(range(HC)))):
                for _ in g:
                    pass
            deferred = iter(())

            # V natural: psum[s, dh] = sum_k vT[k, s] * Wv[k, dh]
            for si in range(NSC):
                ins = []
                for kt in range(KT):
                    t = stream.tile([P, SC], cdt, tag="instream", bufs=12,
                                    name=f"in_v_{si}_{kt}")
                    nc.sync.dma_start(
                        t[:], vT[kt * P:(kt + 1) * P, si * SC:(si + 1) * SC])
                    ins.append(t)
                for sub in range(SC // P):
                    jt_idx = si * (SC // P) + sub
                    ps = psum.tile([P, DH], f32, tag="mm", bufs=4,
                                   name=f"ps_v_{jt_idx}")
                    for kt in range(KT):
                        nc.tensor.matmul(
                            ps[:],
                            lhsT=mm(ins[kt][:, sub * P:(sub + 1) * P]),
                            rhs=mm(wv_sb[:, kt, :]),
                            start=(kt == 0), stop=(kt == KT - 1))
                    for c in range(HC):
                        nc.vector.tensor_add(
                            v_c[c][:, jt_idx, :, 0:DK],
                            ps[:, c * P:(c + 1) * P].rearrange(
                                "p (h d) -> p h d", d=DK),
                            bvb_sb[:, c * HPC:(c + 1) * HPC, :])

            # ---- attention (flash over j, scores transposed) ----
            # Per-head blocks; sc has two buffers so scores(jt+1) overlap
            # exp(jt). AV matmuls trail one j-step so the PE program never
            # blocks the ACT engine behind unready work.
            for h in range(H_CORE):
                hc = h // HPC
                hh = h % HPC
                p0 = hh * DK
                for ibx in range(NIB):
                    i0 = ibx * ib
                    avs = [
                        psum.tile([P, SC], f32, tag="mm", bufs=4,
                                  name=f"av_{h}_{ibx}_{ic}")
                        for ic in range(ICB)
                    ]
                    e_ts = {}
                    for jt in range(JT + 1):
                        if jt < JT:
                            sc_t = psum.tile([P, ib], f32, tag="sc",
                                             bufs=2,
                                             name=f"sc_{h}_{ibx}_{jt}")
                            for ic in range(ICB):
                                nc.tensor.matmul(
                                    sc_t[:, ic * SC:(ic + 1) * SC],
                                    lhsT=mm(kt_h[h][DK:P,
                                                    jt * P:(jt + 1) * P]),
                                    rhs=mm(qt_h[h][DK:P,
                                                   i0 + ic * SC:i0 + (ic + 1) * SC]),
                                    start=True, stop=True)
                            e_t = stream.tile([P, ib], cdt, tag="e", bufs=3,
                                              name=f"e_{h}_{ibx}_{jt}")
                            nc.scalar.activation(e_t[:], sc_t[:], Exp,
                                                 bias=0.0, scale=scale)
                            e_ts[jt] = e_t
                        if jt >= 1:
                            pj = jt - 1
                            e_t = e_ts.pop(pj)
                            for ic in range(ICB):
                                nc.tensor.matmul(
                                    avs[ic][0:DK + 1, :],
                                    lhsT=mm(v_c[hc][:, pj, hh, :]),
                                    rhs=mm(e_t[:, ic * SC:(ic + 1) * SC]),
                                    start=(pj == 0), stop=(pj == JT - 1))
                    # drain AV psums to SBUF, normalize in the background
                    for ic in range(ICB):
                        av = avs[ic]
                        av_sb = stream.tile([P, SC], f32, tag="avsb", bufs=4,
                                            name=f"avsb_{h}_{ibx}_{ic}")
                        nc.vector.tensor_copy(av_sb[0:DK + 1, :],
                                              av[0:DK + 1, :])
                        rz = stream.tile([P, SC], f32, tag="rz", bufs=2,
                                         name=f"rz_{h}_{ibx}_{ic}")
                        nc.vector.reciprocal(rz[DK:DK + 1, :],
                                             av_sb[DK:DK + 1, :])
                        rz_d = dscratch.tile([1, SC], f32, tag="rzd", bufs=2,
                                             name=f"rzd_{h}_{ibx}_{ic}")
                        nc.sync.dma_start(rz_d[:], rz[DK:DK + 1, :])
                        rzb = stream.tile([P, SC], f32, tag="rzb", bufs=2,
                                          name=f"rzb_{h}_{ibx}_{ic}")
                        nc.sync.dma_start(
                            rzb[0:DK, :],
                            rz_d[:, :].to_broadcast((DK, SC)))
                        ot = stream.tile([P, SC], cdt, tag="ot", bufs=2,
                                         name=f"ot_{h}_{ibx}_{ic}")
                        nc.vector.tensor_mul(ot[0:DK, :], av_sb[0:DK, :],
                                             rzb[0:DK, :])
                        nc.sync.dma_start(
                            on_c[hc][p0:p0 + DK,
                                     i0 + ic * SC:i0 + (ic + 1) * SC],
                            ot[0:DK, :])

            # ---- output projection ----
            Ident = mybir.ActivationFunctionType.Identity
            for n in range(NOUT):
                for i in range(NSC):
                    idx = n * NSC + i
                    ps = psum.tile([P, SC], f32, tag=("sc", "mm")[idx % 2],
                                   bufs=(2, 4)[idx % 2],
                                   name=f"ps_o_{n}_{i}")
                    for c in range(HC):
                        nc.tensor.matmul(
                            ps[:],
                            lhsT=mm(wo_sb[:, c, n * P:(n + 1) * P]),
                            rhs=mm(on_c[c][:, i * SC:(i + 1) * SC]),
                            start=(c == 0), stop=(c == HC - 1))
                    o_sb = stream.tile([P, SC], f32, tag="osb", bufs=4,
                                       name=f"o_sb_{n}_{i}")
                    if idx % 2 == 0:
                        nc.scalar.activation(o_sb[:], ps[:], Ident,
                                             bias=bo_sb[:, n:n + 1],
                                             scale=1.0)
                    else:
                        nc.vector.tensor_add(
                            o_sb[:], ps[:],
                            bo_sb[:, n:n + 1].to_broadcast((P, SC)))
                    nc.sync.dma_start(
                        outT[n * P:(n + 1) * P, i * SC:(i + 1) * SC], o_sb[:])

    nc.finalize()
    return nc


def qT_ap_rearr(w_dram, p):
    """[D, N] dram weight -> [P, D//P, N] AP for SBUF load."""
    return w_dram[:, :].rearrange("(ko p) n -> p ko n", p=p)


def make_in_maps(query, key, value, Wq, bq, Wk, bk, Wv, bv, Wo, bo,
                 dtype="f32r"):
    """Shard full inputs into the 8 per-core input dicts."""
    f = lambda a: np.ascontiguousarray(np.asarray(a, dtype=np.float32))
    HC = DH // P
    NOUT = D // P
    query, key, value = f(query), f(key), f(value)
    Wq, Wk, Wv, Wo = f(Wq), f(Wk), f(Wv), f(Wo)
    bq, bk, bv, bo = f(bq), f(bk), f(bv), f(bo)
    if dtype == "bf16":
        import ml_dtypes
        cvt = lambda a: np.ascontiguousarray(a.astype(ml_dtypes.bfloat16))
    else:
        cvt = np.ascontiguousarray
    in_maps = []
    for core in range(N_CORES):
        b, g = core // GROUPS, core % GROUPS
        sl = slice(g * DH, (g + 1) * DH)
        in_maps.append({
            "qT": cvt(query[b].T),
            "kTd": cvt(key[b].T),
            "vT": cvt(value[b].T),
            "wq": cvt(Wq[:, sl]),
            "wk": cvt(Wk[:, sl]),
            "wv": cvt(Wv[:, sl]),
            "wo": cvt(Wo[sl, :]),
            "bq": np.ascontiguousarray(bq[sl].reshape(HC, P).T),
            "bk": np.ascontiguousarray(bk[sl].reshape(HC, P).T),
            "bvb": np.ascontiguousarray(
                np.broadcast_to(bv[sl].reshape(H_CORE, DK)[None], (P, H_CORE, DK))),
            "bo": (np.ascontiguousarray(bo.reshape(NOUT, P).T)
                   if g == 0 else np.zeros((P, NOUT), np.float32)),
        })
    return in_maps


# test hooks (ignored by the harness)
TRACE = False
LAST_RESULT = None
DTYPE = "bf16"
_NC_CACHE = {}


def kernel(query, key, value, Wq, bq, Wk, bk, Wv, bv, Wo, bo):
    global LAST_RESULT
    from concourse.bass_utils import run_bass_kernel_spmd

    if DTYPE not in _NC_CACHE:
        _NC_CACHE[DTYPE] = build_nc(dtype=DTYPE)
    nc = _NC_CACHE[DTYPE]

    in_maps = make_in_maps(query, key, value, Wq, bq, Wk, bk, Wv, bv, Wo, bo,
                           dtype=DTYPE)
    kwargs = {}
    if TRACE:
        kwargs = dict(trace=True, trace_cores=[0])
    res = run_bass_kernel_spmd(nc, in_maps, core_ids=list(range(N_CORES)), **kwargs)
    LAST_RESULT = res

    out = np.zeros((B, S, D), np.float32)
    for core in range(N_CORES):
        b = core // GROUPS
        out[b] += res.results[core]["outT"].T
    return out
